# revision 1
# baseline (speedup 1.0000x reference)
"""Trainium2 Bass kernel for DMSA (distance-modulated multi-head self-attention).

Sharding: 8 cores = 4 batches x 2 head-groups (4 heads each).
Per core (batch b, heads 4g..4g+4):
  qT/kT projections [32/head (+negC row), n], V [n, 32/head + ones col],
  pairwise -sqrt distances via augmented gram matmul + ACT sqrt,
  scores computed transposed with per-row softmax-safety offset negC[q]
  injected through an augmented contraction row, dist*beta mask
  transpose-accumulated into the scores PSUM, one exp (PSUM->SBUF,
  constant bias -scorebound), attn@V with ones-column giving softmax
  denominators, per-head normalization, out_proj partial.
Host sums the two partials per batch and adds out_proj_b.
"""

import sys
import numpy as np

sys.path.insert(0, "/opt/trn_rl_repo")

import concourse.bass as bass
import concourse.mybir as mybir
import concourse.tile as tile
from concourse.masks import make_identity

F32 = mybir.dt.float32
AF = mybir.ActivationFunctionType
ALU = mybir.AluOpType

B, N, D, H, HD = 4, 1200, 256, 8, 32
QT = 120          # q/n tile size
NT = 10           # number of tiles
QPAD = 128 * NT   # padded q layout width (1280)
HEADS = 4         # heads per core
SCALE = HD ** -0.5

_cached = {}

_ev_counter = [0]


def legalize_waits(nc):
    """neuronxcc walrus allows only ONE sync-wait per engine instruction.
    Hoist excess waits onto standalone EventSemaphore instructions inserted
    immediately before the offender (same engine stream, same semantics)."""
    keep_types = ()
    for f in nc.m.functions:
        for bb in f.blocks:
            insts = bb.instructions
            i = 0
            while i < len(insts):
                inst = insts[i]
                si = getattr(inst, "sync_info", None)
                ty = type(inst).__name__
                if si is not None and ty not in keep_types:
                    ow = list(si.on_wait or [])
                    if len(ow) > 1:
                        si.on_wait = ow[-1:]
                        for w in ow[:-1]:
                            _ev_counter[0] += 1
                            ev = mybir.InstEventSemaphore(
                                name=f"legalize_wait_{_ev_counter[0]}")
                            ev.engine = inst.engine
                            ev.sync_info = mybir.SyncInfo(on_wait=[w],
                                                          on_update=[])
                            insts.insert(i, ev)
                            i += 1
                i += 1


def build_kernel():
    nc = bass.Bass()

    def inp(name, shape):
        return nc.declare_dram_parameter(name, list(shape), F32, isOutput=False)

    featT0 = inp("featT0", (128, N))
    featT1 = inp("featT1", (128, N))
    fones = inp("fones", (1, N))
    cts = inp("cts", (3, N))               # (-2x, -2y, 1)
    cta = inp("cta", (3, N))               # (x, y, x^2+y^2)
    cn2p = inp("cn2p", (QT, NT))           # cn2 per-tile cols
    wqt = inp("wqt", (257, 128))           # (Wq.T*scale ; bq*scale)
    wkt = inp("wkt", (257, 128))
    wvt = inp("wvt", (257, 132))           # per head: 32 v cols + ones col
    nbw = inp("nbw", (257, 4))             # -(beta_w.T ; beta_b)
    wot = inp("wot", (128, 256))
    negsb = inp("negsb", (QT, 1))          # -scorebound replicated
    identin = inp("identin", (128, 128))   # identity (transpose stationary)

    fout = nc.declare_dram_parameter("fout", [N, D], F32, isOutput=True)

    with tile.TileContext(nc) as tc:
        with (
            tc.tile_pool(name="persist", bufs=1) as persist,
            tc.tile_pool(name="work", bufs=2) as work,
            tc.tile_pool(name="PS", bufs=1, space="PSUM") as PS,
        ):
            # persistent PSUM tiles; all psum use is manual slicing of these
            # (pool-slot rotation would emit PE self-waits, and a Matmult
            # instruction can carry at most ONE sync wait in this toolchain)
            PA = PS.tile([128, 2048], F32, name="PA")  # 4 banks
            PB = PS.tile([128, 1536], F32, name="PB")  # 3 banks
            PC = PS.tile([128, 512], F32, name="PC")   # 1 bank

            ident = persist.tile([128, 128], F32)
            wot_sb = persist.tile([128, 256], F32)
            negsb_sb = persist.tile([QT, 1], F32)
            ones128 = persist.tile([1, 128], F32)
            nc.vector.memset(ones128[:], 1.0)
            sums_sb = persist.tile([1, N], F32)
            # partition-0 row: ones (stage 1) then recip (stage 3)
            onesrec = persist.tile([1, N], F32)

            qT = [persist.tile([33, QPAD], F32, name=f"qT{h}") for h in range(HEADS)]
            kT = [persist.tile([33, N], F32, name=f"kT{h}") for h in range(HEADS)]
            V = persist.tile([QT, NT * 132], F32)
            nb = persist.tile([QT, NT * 4], F32)
            nbr = persist.tile([QT, NT * 4], F32)
            ndist = persist.tile([128, NT * N], F32)
            mega = persist.tile([QT, NT * QPAD], F32)
            outT = persist.tile([128, N], F32)
            outT_n = persist.tile([128, N], F32)
            cts_sb = persist.tile([3, N], F32)
            cta_sb = persist.tile([3, N], F32)
            maskt = [persist.tile([QT, N], F32, name=f"maskt{ii}")
                     for ii in range(5)]
            pcr = persist.tile([1, 2], F32)

            for h in range(HEADS):
                nc.vector.memset(kT[h][32:33, :], 1.0)
                nc.vector.memset(qT[h][:], 0.0)   # pad cols read by scores MMs
            nc.vector.memset(mega[:], 0.0)        # pad cols read by attnV MMs

            # ============ stage 1: loads (staged through mega) + projections
            # Every DRAM input is DMA'd into spare mega columns, then copied
            # to its real tile by the DVE. Wide DMAs fan out over several
            # HW-DGE queues; routing all completions through DVE copies keeps
            # multi-queue waits OFF the matmuls (which allow only one wait).
            with tc.tile_pool(name="s1c", bufs=1) as s1c:
                # fa0/fa1 live inside ndist's first columns (freed when the
                # sqrt pass overwrites them later -- WAR handled by Tile)
                fa0 = ndist[0:128, 0:N]
                fa1 = ndist[0:128, N:2 * N]
                wqt_sb = s1c.tile([128, 2 * 128], F32)
                wkt_sb = s1c.tile([128, 2 * 128], F32)
                wvt_sb = s1c.tile([128, 2 * 132], F32)
                nbw_sb = s1c.tile([128, 2 * 4], F32)
                wqb_sb = s1c.tile([1, 128], F32)
                wkb_sb = s1c.tile([1, 128], F32)
                wvb_sb = s1c.tile([1, 132], F32)
                nbb_sb = s1c.tile([1, 4], F32)
                cn2_sb = s1c.tile([QT, NT], F32)

                stage = [2 * N]
                def load(dst_ap, src_ap, pwidth, cols):
                    # stage through ndist columns: PE never reads ndist, so
                    # the multi-queue DMA waits stay on DVE/ACT consumers
                    c0 = stage[0]
                    sl = ndist[0:pwidth, c0:c0 + cols]
                    nc.sync.dma_start(out=sl, in_=src_ap)
                    nc.vector.tensor_copy(dst_ap, sl)
                    stage[0] = c0 + cols

                load(fa0[:], featT0[:], 128, N)
                load(fa1[:], featT1[:], 128, N)
                for dst, dstb, srct, w in (
                    (wqt_sb, wqb_sb, wqt, 128),
                    (wkt_sb, wkb_sb, wkt, 128),
                    (wvt_sb, wvb_sb, wvt, 132),
                    (nbw_sb, nbb_sb, nbw, 4),
                ):
                    load(dst[:, 0:w], srct[0:128, :], 128, w)
                    load(dst[:, w:2 * w], srct[128:256, :], 128, w)
                    load(dstb[:], srct[256:257, :], 1, w)
                load(onesrec[:], fones[:], 1, N)
                load(wot_sb[:], wot[:], 128, 256)
                load(negsb_sb[:], negsb[:], QT, 1)
                load(cts_sb[:], cts[:], 3, N)
                load(cta_sb[:], cta[:], 3, N)
                load(cn2_sb[:], cn2p[:], QT, NT)
                load(ident[:], identin[:], 128, 128)

                # qT/kT: chunks at 512-col (bank-aligned) offsets
                for (wsb, wbias, dsts, PX) in (
                    (wqt_sb, wqb_sb, qT, PA),
                    (wkt_sb, wkb_sb, kT, PB),
                ):
                    for c, (c0, cw) in enumerate(((0, 480), (480, 480),
                                                  (960, 240))):
                        ps = PX[0:128, 512 * c:512 * c + cw]
                        for ci in range(3):
                            if ci < 2:
                                lhsT = wsb[:, 128 * ci:128 * ci + 128]
                                rhs = (fa0 if ci == 0 else fa1)[:, c0:c0 + cw]
                            else:
                                lhsT = wbias[:]
                                rhs = onesrec[:, c0:c0 + cw]
                            nc.tensor.matmul(ps, lhsT, rhs,
                                             start=(ci == 0), stop=(ci == 2))
                        for h in range(HEADS):
                            if dsts is qT:
                                nblk = cw // QT
                                src_ = ps[32 * h:32 * h + 32, 0:cw].rearrange(
                                    "p (t q) -> p t q", q=QT)
                                base = (c0 // QT) * 128
                                dst_ = dsts[h][0:32,
                                               base:base + 128 * nblk].rearrange(
                                    "p (t q) -> p t q", q=128)[:, :, 0:QT]
                                nc.scalar.copy(dst_, src_)
                            else:
                                nc.scalar.copy(dsts[h][0:32, c0:c0 + cw],
                                               ps[32 * h:32 * h + 32, 0:cw])

                # V + negbeta per n-tile (disjoint psum columns)
                for i in range(NT):
                    vc = 512 * (i // 3) + 132 * (i % 3)  # bank-safe slot
                    pv = PA[0:QT, vc:vc + 132]
                    pb = PB[0:QT, 132 * i:132 * i + 4]
                    for ci in range(3):
                        if ci < 2:
                            lhsT = (fa0 if ci == 0 else fa1)[:, QT * i:QT * i + QT]
                            rhs_v = wvt_sb[:, 132 * ci:132 * ci + 132]
                            rhs_b = nbw_sb[:, 4 * ci:4 * ci + 4]
                        else:
                            lhsT = onesrec[:, QT * i:QT * i + QT]
                            rhs_v = wvb_sb[:]
                            rhs_b = nbb_sb[:]
                        nc.tensor.matmul(pv, lhsT, rhs_v,
                                         start=(ci == 0), stop=(ci == 2))
                        nc.tensor.matmul(pb, lhsT, rhs_b,
                                         start=(ci == 0), stop=(ci == 2))
                    nc.scalar.copy(V[:, 132 * i:132 * i + 132], pv)
                    nc.scalar.copy(nb[:, 4 * i:4 * i + 4], pb)

                nc.scalar.activation(nbr[:], nb[:], AF.Relu)

                # ============ stage 2: distances ============
                # absorb the fresh DVE tick (cts/cta copies) into the PE
                # clock so the gram MMs only carry the WAR-vs-evict (ACT)
                nc.tensor.matmul(PC[0:1, 404:405], ident[0:1, 0:1],
                                 ident[0:1, 0:1], start=True, stop=True)
                nc.vector.tensor_copy(pcr[0:1, 0:1], PC[0:1, 404:405])

                for i in range(NT):
                    pg = PB[0:QT, 0:N]
                    for c0, cw in ((0, 512), (512, 512), (1024, 176)):
                        nc.tensor.matmul(pg[:, c0:c0 + cw],
                                         cts_sb[:, QT * i:QT * i + QT],
                                         cta_sb[:, c0:c0 + cw],
                                         start=True, stop=True)
                    d2c = work.tile([QT, N], F32, tag="d2c", bufs=1)
                    nc.vector.tensor_scalar(d2c[:], pg[:, 0:N],
                                            cn2_sb[:, i:i + 1], 0.0,
                                            op0=ALU.add, op1=ALU.max)
                    nc.scalar.activation(ndist[0:QT, N * i:N * i + N], d2c[:],
                                         AF.Sqrt)
                    ndm = work.tile([QT, 1], F32, tag="ndm")
                    nc.vector.tensor_reduce(ndm[:], ndist[0:QT, N * i:N * i + N],
                                            op=ALU.max, axis=mybir.AxisListType.X)
                    # negC cols [120,4] = -relu(negbeta) * ndistmax
                    ngc = work.tile([QT, 4], F32, tag="ngc")
                    nc.vector.tensor_scalar(ngc[:], nbr[:, 4 * i:4 * i + 4],
                                            ndm[:], -1.0,
                                            op0=ALU.mult, op1=ALU.mult)
                    for h in range(HEADS):
                        ptq = PC[0:1, 0:QT]
                        nc.tensor.transpose(ptq, ngc[:, h:h + 1],
                                            ident[0:QT, 0:QT])
                        nc.vector.tensor_copy(
                            qT[h][32:33, 128 * i:128 * i + QT], ptq)

                # ladder: absorb the final DVE tick before stage 3
                nc.tensor.matmul(
                    PC[0:1, 408:409],
                    qT[3][32:33, 128 * (NT - 1):128 * (NT - 1) + 1],
                    qT[3][32:33, 128 * (NT - 1):128 * (NT - 1) + 1],
                    start=True, stop=True)
                nc.vector.tensor_copy(pcr[0:1, 1:2], PC[0:1, 408:409])
                # pre-touch both pk double-buffer regions so the first
                # stage-3 writers inherit no cross-stage WAW waits
                nc.tensor.matmul(PA[0:1, 0:1], ident[0:1, 0:1],
                                 ident[0:1, 0:1], start=True, stop=True)
                nc.tensor.matmul(PA[0:1, 1024:1025], ident[0:1, 0:1],
                                 ident[0:1, 0:1], start=True, stop=True)

            # ============ stage 3: attention ============
            if True:
                jj = 0
                for h in range(HEADS):
                    for half in range(2):
                        i0 = 5 * half
                        masks = []
                        for ii in range(5):
                            i = i0 + ii
                            m = maskt[ii]
                            nc.scalar.mul(m[:], ndist[0:QT, N * i:N * i + N],
                                          nb[:, 4 * i + h:4 * i + h + 1])
                            masks.append(m)
                        for j in range(NT):
                            pk = PA[0:QT, 1024 * (jj % 2):1024 * (jj % 2) + 640]
                            jj += 1
                            qbase = 640 * half
                            nc.tensor.matmul(pk[:, 0:512],
                                             kT[h][:, QT * j:QT * j + QT],
                                             qT[h][:, qbase:qbase + 512],
                                             start=True, stop=False,
                                             skip_group_check=True)
                            nc.tensor.matmul(pk[:, 512:640],
                                             kT[h][:, QT * j:QT * j + QT],
                                             qT[h][:, qbase + 512:qbase + 640],
                                             start=True, stop=False,
                                             skip_group_check=True)
                            for ii in range(5):
                                nc.tensor.matmul(
                                    pk[:, 128 * ii:128 * ii + QT],
                                    masks[ii][:, QT * j:QT * j + QT],
                                    ident[0:QT, 0:QT],
                                    is_transpose=True, start=False,
                                    stop=(ii == 4), skip_group_check=True)
                            src_ = pk.rearrange("p (t q) -> p t q",
                                                q=128)[:, :, 0:QT]
                            d0 = QPAD * j + qbase
                            dst_ = mega[:, d0:d0 + 640].rearrange(
                                "p (t q) -> p t q", q=128)[:, :, 0:QT]
                            nc.scalar.activation(dst_, src_, AF.Exp,
                                                 bias=negsb_sb[:, 0:1])
                    # attnV with ones-column -> sums in row 32
                    po = PB[0:33, 0:QPAD]
                    for j in range(NT):
                        for c0, cw in ((0, 512), (512, 512), (1024, 256)):
                            nc.tensor.matmul(
                                po[:, c0:c0 + cw],
                                V[:, 132 * j + 33 * h:132 * j + 33 * h + 33],
                                mega[:, QPAD * j + c0:QPAD * j + c0 + cw],
                                start=(j == 0), stop=(j == NT - 1))
                    srcs = po[0:32, :].rearrange("p (t q) -> p t q",
                                                 q=128)[:, :, 0:QT]
                    nc.scalar.copy(outT[32 * h:32 * h + 32, :].rearrange(
                        "p (t q) -> p t q", q=QT), srcs)
                    srow = po[32:33, :].rearrange("p (t q) -> p t q",
                                                  q=128)[:, :, 0:QT]
                    nc.scalar.copy(sums_sb[:].rearrange("p (t q) -> p t q",
                                                        q=QT), srow)
                    nc.vector.reciprocal(onesrec[:], sums_sb[:])
                    # normalize: PE broadcast of recip into PC, DVE multiply
                    for c0, cw in ((0, 512), (512, 512), (1024, 176)):
                        prb = PC[0:128, 0:cw]
                        nc.tensor.matmul(prb, ones128[:],
                                         onesrec[:, c0:c0 + cw],
                                         start=True, stop=True)
                        nc.vector.tensor_tensor(
                            out=outT_n[32 * h:32 * h + 32, c0:c0 + cw],
                            in0=outT[32 * h:32 * h + 32, c0:c0 + cw],
                            in1=prb[32 * h:32 * h + 32, 0:cw], op=ALU.mult)

                # ============ stage 4: out_proj partial ============
                for t in range(NT):
                    pf = PC[0:QT, 256 * (t % 2):256 * (t % 2) + 256]
                    nc.tensor.matmul(pf, outT_n[:, QT * t:QT * t + QT],
                                     wot_sb[:], start=True, stop=True)
                    fo_sb = work.tile([QT, 256], F32, tag="fo")
                    nc.scalar.copy(fo_sb[:], pf)
                    nc.sync.dma_start(out=fout[QT * t:QT * t + QT, :],
                                      in_=fo_sb[:])

    return nc


def _host_prep(inputs):
    feat = np.ascontiguousarray(inputs["query_feat"], dtype=np.float32)
    bbox = np.ascontiguousarray(inputs["query_bbox"], dtype=np.float32)
    beta_w = np.asarray(inputs["beta_w"], np.float32)
    beta_b = np.asarray(inputs["beta_b"], np.float32)
    ipw = np.asarray(inputs["in_proj_w"], np.float32)
    ipb = np.asarray(inputs["in_proj_b"], np.float32)
    opw = np.asarray(inputs["out_proj_w"], np.float32)

    in_maps = []
    for c in range(8):
        b, g = c // 2, c % 2
        f = feat[b]
        x = bbox[b, :, 0].astype(np.float32)
        y = bbox[b, :, 1].astype(np.float32)
        cn2 = x * x + y * y
        Wq = ipw[0:256][128 * g:128 * g + 128]
        Wk = ipw[256:512][128 * g:128 * g + 128]
        Wv = ipw[512:768][128 * g:128 * g + 128]
        bq = ipb[0:256][128 * g:128 * g + 128]
        bk = ipb[256:512][128 * g:128 * g + 128]
        bv = ipb[512:768][128 * g:128 * g + 128]

        wqt_ = np.concatenate([Wq.T * SCALE, (bq * SCALE)[None, :]], 0)
        wkt_ = np.concatenate([Wk.T, bk[None, :]], 0)
        wvt_ = np.zeros((257, 132), np.float32)
        for hh in range(4):
            wvt_[0:256, 33 * hh:33 * hh + 32] = Wv.T[:, 32 * hh:32 * hh + 32]
            wvt_[256, 33 * hh:33 * hh + 32] = bv[32 * hh:32 * hh + 32]
            wvt_[256, 33 * hh + 32] = 1.0
        nbw_ = np.concatenate([-beta_w[4 * g:4 * g + 4].T,
                               -beta_b[4 * g:4 * g + 4][None, :]], 0)
        qf = f @ Wq.T + bq
        kf = f @ Wk.T + bk
        qn = np.linalg.norm(qf.reshape(N, 4, HD), axis=2).max(0)
        kn = np.linalg.norm(kf.reshape(N, 4, HD), axis=2).max(0)
        sb = float((qn * kn).max()) * SCALE + 1.0

        cn2p_ = cn2.reshape(NT, QT).T.copy()

        in_maps.append({
            "featT0": np.ascontiguousarray(f.T[0:128]),
            "featT1": np.ascontiguousarray(f.T[128:256]),
            "fones": np.ones((1, N), np.float32),
            "cts": np.ascontiguousarray(
                np.stack([-2 * x, -2 * y, np.ones(N, np.float32)])),
            "cta": np.ascontiguousarray(np.stack([x, y, cn2])),
            "cn2p": np.ascontiguousarray(cn2p_),
            "wqt": np.ascontiguousarray(wqt_),
            "wkt": np.ascontiguousarray(wkt_),
            "wvt": np.ascontiguousarray(wvt_),
            "nbw": np.ascontiguousarray(nbw_),
            "wot": np.ascontiguousarray(opw[:, 128 * g:128 * g + 128].T),
            "negsb": np.full((QT, 1), -sb, np.float32),
            "identin": np.eye(128, dtype=np.float32),
        })
    return in_maps


def kernel(**inputs):
    from concourse.bass_utils import run_bass_kernel_spmd

    if "nc" not in _cached:
        nc = build_kernel()
        legalize_waits(nc)
        _cached["nc"] = nc
    nc = _cached["nc"]

    in_maps = _host_prep(inputs)
    res = run_bass_kernel_spmd(nc, in_maps, list(range(8))).results

    opb = np.asarray(inputs["out_proj_b"], np.float32)
    out = np.zeros((B, N, D), np.float32)
    for b in range(B):
        out[b] = res[2 * b]["fout"] + res[2 * b + 1]["fout"] + opb
    return out


if __name__ == "__main__":
    inp = np.load("/tmp/inputs.npy", allow_pickle=True).item()
    out = kernel(**inp)
    ref = np.load("/tmp/ref.npy")
    e = np.abs(out - ref)
    print("max scaled:", (e / np.abs(ref).max()).max(),
          "fro rel:", np.linalg.norm(e) / np.linalg.norm(ref))

